# revision 31
# baseline (speedup 1.0000x reference)
"""MoE layer (router + top-k dispatch + per-expert FFN + weighted combine)
on 8 Trainium2 NeuronCores.

Sharding strategy (hidden-dim sharding = perfectly balanced expert
parallelism):
  - The host computes the router (x @ Wg -> softmax -> top-k) and builds
    ONE global pair list: all routed (token, expert) pairs, grouped by
    expert, each expert's list padded to a multiple of 8.  Every core
    walks the SAME pair list (same program, same chunk structure - clean
    SPMD), but core s only owns the s-th H/8 = 512-wide slice of every
    expert's FFN: W1[e][:, s*512:(s+1)*512] and W2[e][s*512:(s+1)*512, :].
    ReLU is elementwise in H, so slicing the hidden dim is exact; each
    core produces a partial y and the host sums the 8 partials during the
    weighted combine.  Per-core work is exactly sum(counts)/8 = B*k/8
    token-equivalents regardless of how unbalanced the expert counts are
    (expert-per-core would be bound by max(counts)).
  - Device output yT[p, mo, c] (fp16 partials) is summed + combined with
    the fp32 softmax weights host-side (the "weighted return" half of the
    expert-parallel all-to-all).

Device program: per 512-token chunk, GEMM1 (4 psum groups x 8 k-matmuls)
-> relu+bias eviction to bf16 h, GEMM2 (8 psum groups x 4 k-matmuls) ->
bias eviction to fp16 ob -> one batched DMA to DRAM.  The two GEMMs are
software-pipelined one chunk deep (PE order G1(0), G1(1), G2(0), G1(2),
G2(1), ...) so GEMM2 never waits on the tail h eviction of its own
chunk's GEMM1 and the cold-start w2 deadline is one extra chunk out.

Layouts: host pre-permutes everything into flat per-partition consumption
order (w1 as [128p, e, mh, kd, 128q], w2 as [128p, e, mo, kh, 128q], x as
[128p, chunk, kd, col]) so every DMA is a flat contiguous slice arriving
in consumption order.  Weight streaming beyond expert 0 is paced one
segment ahead with a data-dependency gate (a 1-element op reading the
pacing chunk's h and writing into the DMA's destination) — the tile
scheduler hoists dep-free DMA triggers to the top of their queue, so
queue position alone cannot hold a transfer back.  Stores ride the
scalar hardware-DGE queue; x rides sync; the gpsimd software-DGE queue
only carries cold-start weights, paced weight prefetch, and tail stores.

Compute is bf16 (fp32 PSUM accumulation); combine weights stay fp32;
partial y ships as fp16.
"""

import numpy as np
import ml_dtypes
import bass_rust

import concourse.bass as bass
import concourse.mybir as mybir
import concourse.tile as tile
from concourse.bass_utils import run_bass_kernel_spmd

P = 128
N_CORES = 8
CHUNK = 512
MIN_CHUNK = 256  # below ~230 cols LDWEIGHTS (97ns) outruns the matmul


def _normalize_sync_waits(nc):
    """The walrus build in this toolchain rejects >1 sync wait on a single
    instruction (setupSyncWait: "Too many sync wait commands"), while Tile's
    semaphore assignment freely emits several. Hoist all but one wait of each
    instruction onto same-engine NOPs placed immediately before it — the
    engine stream is in-order, so stalling at the NOPs is semantically
    identical to a multi-wait instruction."""
    count = 0
    for f in nc.m.functions:
        for bb in f.blocks:
            out = []
            changed = False
            for ins in bb.instructions:
                si = ins.sync_info
                if si is not None and len(si.on_wait) > 1:
                    waits = list(si.on_wait)
                    for w in waits[:-1]:
                        count += 1
                        out.append(
                            mybir.InstNoOp(
                                name=f"I-nw{count}",
                                ins=[],
                                outs=[],
                                engine=ins.engine,
                                sync_info=bass_rust.SyncInfo(
                                    on_wait=[w], on_update=[]
                                ),
                            )
                        )
                    ins.sync_info = bass_rust.SyncInfo(
                        on_wait=[waits[-1]], on_update=list(si.on_update)
                    )
                    changed = True
                out.append(ins)
            if changed:
                bb.instructions = out
    return nc


def _build_program(D, HL, O, E, chunks, CT):
    """chunks: list of (expert, N, c0) covering [0, CT) in order."""
    f32, bf16, f16 = mybir.dt.float32, mybir.dt.bfloat16, mybir.dt.float16
    KD, MHL, MO = D // P, HL // P, O // P
    AF = mybir.ActivationFunctionType
    G1 = KD * P        # w1 cols per GEMM1 group
    W1E = MHL * G1     # w1 cols per expert slice
    W2E = MO * MHL * P # w2 cols per expert slice

    nc = bass.Bass()
    xTp = nc.declare_dram_parameter("xTp", [P, KD * CT], bf16, isOutput=False)
    w1p = nc.declare_dram_parameter("w1p", [P, E * W1E], bf16, isOutput=False)
    w2p = nc.declare_dram_parameter("w2p", [P, E * W2E], bf16, isOutput=False)
    b1p = nc.declare_dram_parameter("b1p", [P, E * MHL], f32, isOutput=False)
    b2p = nc.declare_dram_parameter("b2p", [P, E * MO], f32, isOutput=False)
    # Output blocks are chunk-contiguous ([mo, col] within each chunk's
    # MO*N-wide block) so every store is one flat 8KB-per-partition-row DMA
    # on a hardware-DGE queue - the strided [P, MO, CT] layout fed through
    # the gpsimd software-DGE queue ran at ~50 GB/s and stalled the whole
    # pipeline through the ob WAR chain.
    yT = nc.declare_dram_parameter("yT", [P, MO * CT], f16, isOutput=True)

    # order of first appearance of each expert in the chunk list
    seg_experts = []
    for e, _, _ in chunks:
        if e not in seg_experts:
            seg_experts.append(e)
    first_chunk_of = {}
    n_chunks_of = {}
    for ci, (e, _, _) in enumerate(chunks):
        if e not in first_chunk_of:
            first_chunk_of[e] = ci
        n_chunks_of[e] = n_chunks_of.get(e, 0) + 1
    # Chunk at which to prefetch the NEXT segment's weights: two chunks into
    # the previous segment (capped at its last chunk) — far enough from the
    # cold-start window that the trigger doesn't steal wire from chunk 0-1's
    # x/w stream, and still >=1.5 chunk periods (~20us) of lead for ~6us of
    # wire.
    # prefetch_at: chunk index -> list of (expert, w1?) gates to emit there.
    # w1 (needed first) gets one chunk more lead than w2; both are paced by
    # a data-dependency gate so jitter headroom is ~2-4 chunk periods.
    prefetch_at = {}
    # expert 0's own w2 is also gated (on chunk 0's first h): it is not
    # consumed until GEMM2(0) at ~+10us, and pulling its 1MB out of the
    # cold window frees wire for the chunk 0-2 x / w1 stream.
    if seg_experts:
        prefetch_at.setdefault(0, []).append((seg_experts[0], False))
    for j in range(1, len(seg_experts)):
        ep = seg_experts[j - 1]
        en = seg_experts[j]
        ci_w1 = first_chunk_of[ep] + min(1, n_chunks_of[ep] - 1)
        ci_w2 = first_chunk_of[ep] + min(2, n_chunks_of[ep] - 1)
        prefetch_at.setdefault(ci_w1, []).append((en, True))
        prefetch_at.setdefault(ci_w2, []).append((en, False))

    with tile.TileContext(nc) as tc:
        with (
            tc.tile_pool(name="sb", bufs=1) as pool,
            tc.tile_pool(name="ps", bufs=4, space="PSUM") as psp,
        ):
            w1_sb = pool.tile([P, E * W1E], bf16)
            w2_sb = pool.tile([P, E * W2E], bf16)
            b1_sb = pool.tile([P, E * MHL], f32)
            b2_sb = pool.tile([P, E * MO], f32)

            e0, N0, _ = chunks[0]

            # Hoist the scalar engine's one-time ACT_TABLE_LOAD (~1.3us) into
            # the cold DMA window with a dependency-free dummy activation.
            dum_i = pool.tile([P, 1], f32, tag="dum_i")
            dum_o = pool.tile([P, 1], f32, tag="dum_o")
            nc.vector.memset(dum_i[:], 0.0)
            nc.scalar.activation(dum_o[:], dum_i[:], AF.Relu)
            # Warm the PE p-state during the cold DMA window: ~3us of dummy
            # matmuls (dep-free, so the scheduler runs them immediately)
            # bring the clock to max before the first real matmul — a cold
            # PE runs the first ~3us of matmuls at 0.65-1.2GHz instead of
            # 2.4GHz.
            dum_w = pool.tile([P, P], bf16, tag="dum_w")
            dum_x = pool.tile([P, CHUNK], bf16, tag="dum_x")
            nc.vector.memset(dum_w[:], 0.0)
            nc.vector.memset(dum_x[:], 0.0)
            dum_ps = psp.tile([P, CHUNK], f32, tag="ph", bufs=3)
            for i in range(11):
                nc.tensor.matmul(
                    dum_ps[:], dum_w[:], dum_x[:],
                    start=(i == 0), stop=(i == 10),
                )

            # Cold start: b1 first (needed at the first eviction), then the
            # chunk-0 x slices and expert-0 weights as flat consumption-order
            # slices.  x rides the sync queue, weights the gpsimd queue; the
            # two streams split the wire, and chunk-0's GEMM1 group g only
            # waits on its own slice (Tile RAW deps are per-DMA-region), so
            # w1 goes group-by-group.
            nc.scalar.dma_start(b1_sb[:], b1p[:])
            xc0 = pool.tile([P, KD * CHUNK], bf16, tag="xc", bufs=2)
            # finest-grain first slices: MM0 only needs x[kd0] + w1[g0,kd0]
            nc.sync.dma_start(xc0[:, :N0], xTp[:, :N0])
            nc.gpsimd.dma_start(
                w1_sb[:, e0 * W1E : e0 * W1E + P],
                w1p[:, e0 * W1E : e0 * W1E + P],
            )
            nc.sync.dma_start(xc0[:, N0 : 3 * N0], xTp[:, N0 : 3 * N0])
            nc.gpsimd.dma_start(
                w1_sb[:, e0 * W1E + P : e0 * W1E + G1],
                w1p[:, e0 * W1E + P : e0 * W1E + G1],
            )
            nc.sync.dma_start(xc0[:, 3 * N0 : 5 * N0], xTp[:, 3 * N0 : 5 * N0])
            nc.sync.dma_start(xc0[:, 5 * N0 : KD * N0], xTp[:, 5 * N0 : KD * N0])
            for g in range(1, MHL):
                nc.gpsimd.dma_start(
                    w1_sb[:, e0 * W1E + g * G1 : e0 * W1E + (g + 1) * G1],
                    w1p[:, e0 * W1E + g * G1 : e0 * W1E + (g + 1) * G1],
                )
            nc.scalar.dma_start(b2_sb[:], b2p[:])

            xcs = {0: xc0}
            hs = {}
            last_ci = len(chunks) - 1

            def emit_g1(ci):
                e, N, c0 = chunks[ci]
                # prefetch next chunk's x (sync queue; self-paced by the
                # bufs=2 WAR dependency after the first two chunks)
                if ci + 1 < len(chunks):
                    e1, N1, c1 = chunks[ci + 1]
                    xn = pool.tile([P, KD * CHUNK], bf16, tag="xc", bufs=2)
                    nc.sync.dma_start(
                        xn[:, : KD * N1], xTp[:, KD * c1 : KD * (c1 + N1)]
                    )
                    xcs[ci + 1] = xn
                xc = xcs.pop(ci)
                pf = prefetch_at.get(ci)
                h_a = pool.tile([P, MHL - MHL // 2, CHUNK], bf16, tag="h_a", bufs=2)
                h_b = pool.tile([P, MHL // 2, CHUNK], bf16, tag="h_b", bufs=2)
                hs[ci] = (h_a, h_b)

                def h_slice(kh, N=N, h_a=h_a, h_b=h_b):
                    half = MHL - MHL // 2
                    t = h_a if kh < half else h_b
                    return t[:, kh if kh < half else kh - half, :N]

                for mh in range(MHL):
                    ph = psp.tile([P, CHUNK], f32, tag="ph", bufs=3)
                    for kd in range(KD):
                        u = (e * MHL + mh) * KD + kd
                        nc.tensor.matmul(
                            ph[:, :N],
                            w1_sb[:, u * P : (u + 1) * P],
                            xc[:, kd * N : (kd + 1) * N],
                            start=(kd == 0),
                            stop=(kd == KD - 1),
                        )
                    # h eviction on the otherwise-idle DVE (relu(ph + b1) via
                    # tensor_scalar) keeps the scalar queue for GEMM2
                    # evictions + DMA triggers — scalar saturation here
                    # stalled GEMM2 at segment boundaries.
                    nc.vector.tensor_scalar(
                        h_slice(mh), ph[:, :N],
                        b1_sb[:, e * MHL + mh : e * MHL + mh + 1], 0.0,
                        mybir.AluOpType.add, mybir.AluOpType.max,
                    )
                    if pf is not None and mh == 0:
                        # Pace the next segment's weight stream behind THIS
                        # chunk's compute with a real data dependency — the
                        # tile scheduler freely hoists dep-free DMA triggers
                        # to the top of their queue (observed: all prefetches
                        # fired inside the cold window, starving chunk 0-1's
                        # x stream).  A 1-element gate op reads this chunk's
                        # freshly-evicted h (RAW on compute) and writes into
                        # each DMA's destination region (WAW with the DMA).
                        for en, is_w1 in pf:
                            wsb, wpr, sz = (
                                (w1_sb, w1p, W1E) if is_w1 else (w2_sb, w2p, W2E)
                            )
                            nc.vector.tensor_scalar(
                                wsb[:, en * sz : en * sz + 1],
                                h_slice(0)[:, 0:1], 0.0, None,
                                mybir.AluOpType.mult,
                            )
                            nc.gpsimd.dma_start(
                                wsb[:, en * sz : (en + 1) * sz],
                                wpr[:, en * sz : (en + 1) * sz],
                            )

            def emit_g2(ci):
                e, N, c0 = chunks[ci]
                h_a, h_b = hs.pop(ci)
                half = MHL - MHL // 2

                def h_slice(kh):
                    t = h_a if kh < half else h_b
                    return t[:, kh if kh < half else kh - half, :N]

                ob = pool.tile([P, MO, CHUNK], f16, tag="ob", bufs=4)
                for mo in range(MO):
                    py = psp.tile([P, CHUNK], f32, tag="py", bufs=5)
                    for kh in range(MHL):
                        u = (e * MO + mo) * MHL + kh
                        nc.tensor.matmul(
                            py[:, :N],
                            w2_sb[:, u * P : (u + 1) * P],
                            h_slice(kh),
                            start=(kh == 0),
                            stop=(kh == MHL - 1),
                        )
                    if ci >= last_ci - 1:
                        # tail-critical final chunks: evictions move to the
                        # end-idle Vector engine (bias-add via tensor_scalar)
                        # so the scalar queue only carries store triggers,
                        # and each mo group streams out immediately on the
                        # sync/scalar hardware-DGE queues (gpsimd SW-DGE
                        # costs ~1us fixed per store and dominated the
                        # drain; extra triggers on a busy scalar stalled
                        # the PE before that).
                        nc.vector.tensor_scalar(
                            ob[:, mo, :N], py[:, :N],
                            b2_sb[:, e * MO + mo : e * MO + mo + 1], None,
                            mybir.AluOpType.add,
                        )
                        eng = nc.sync if mo % 2 == 0 else nc.scalar
                        eng.dma_start(
                            yT[:, MO * c0 + mo * N : MO * c0 + (mo + 1) * N],
                            ob[:, mo, :N],
                        )
                    else:
                        nc.scalar.activation(
                            ob[:, mo, :N], py[:, :N], AF.Identity,
                            bias=b2_sb[:, e * MO + mo : e * MO + mo + 1],
                        )
                if ci < last_ci - 1:
                    # one flat store per chunk (scalar queue = hardware DGE,
                    # naturally ordered after the mo=7 eviction)
                    nc.scalar.dma_start(
                        yT[:, MO * c0 : MO * (c0 + N)], ob[:, :, :N]
                    )

            # software pipeline: G1(0), G1(1), G2(0), G1(2), G2(1), ...
            emit_g1(0)
            for ci in range(len(chunks)):
                if ci + 1 < len(chunks):
                    emit_g1(ci + 1)
                emit_g2(ci)
    return _normalize_sync_waits(nc)


def _chunk_counts(c):
    """Split a padded per-expert count into chunks <=CHUNK, all >=MIN_CHUNK
    (borrowing from the previous full chunk when the tail is short)."""
    ch = [CHUNK] * (c // CHUNK)
    rem = c % CHUNK
    if rem:
        if rem < MIN_CHUNK and ch:
            ch[-1] -= MIN_CHUNK - rem
            rem = MIN_CHUNK
        ch.append(rem)
    return ch


def kernel(**inputs):
    x = np.ascontiguousarray(np.asarray(inputs["x"], dtype=np.float32))
    Wg = np.ascontiguousarray(np.asarray(inputs["Wg"], dtype=np.float32))
    W1 = np.asarray(inputs["W1"], dtype=np.float32)
    b1 = np.asarray(inputs["b1"], dtype=np.float32)
    W2 = np.asarray(inputs["W2"], dtype=np.float32)
    b2 = np.asarray(inputs["b2"], dtype=np.float32)
    k = int(np.asarray(inputs["k"]))

    B, D = x.shape
    E = Wg.shape[1]
    H = W1.shape[2]
    O = W2.shape[2]
    HL = H // N_CORES              # hidden slice per core
    KD, MHL, MO = D // P, HL // P, O // P

    # Host-side router: softmax probs (combine weights) and top-k expert
    # membership (softmax is monotonic, so top-k on logits == on probs).
    logits = x @ Wg
    m = logits.max(axis=1, keepdims=True)
    p = np.exp(logits - m)
    probs = p / p.sum(axis=1, keepdims=True)
    kth = np.partition(logits, E - k, axis=1)[:, E - k]
    routed = logits >= kth[:, None]
    idx_per_e = [np.nonzero(routed[:, e])[0] for e in range(E)]
    counts = [len(ix) for ix in idx_per_e]
    cpad = [-(-c // 2) * 2 for c in counts]

    # Global chunk list over the pair list (grouped by expert).  The first
    # two chunks are 256-col (halves the cold-start x wire ahead of the
    # first GEMMs) and the global last chunk is 256-col (shrinks the tail
    # store drain).
    active = [e for e in range(E) if cpad[e] > 0]
    chunks = []   # (expert, N, global col offset)
    exp_off = [0] * E
    CT = 0
    for e in range(E):
        exp_off[e] = CT
        c = cpad[e]
        if active and e == active[0] and c >= CHUNK + 2 * MIN_CHUNK:
            ch = [MIN_CHUNK, MIN_CHUNK] + _chunk_counts(c - 2 * MIN_CHUNK)
        elif active and e == active[-1] and c >= CHUNK + 3 * MIN_CHUNK:
            ch = _chunk_counts(c - 2 * MIN_CHUNK) + [MIN_CHUNK, MIN_CHUNK]
        else:
            ch = _chunk_counts(c)
        for N in ch:
            chunks.append((e, N, CT))
            CT += N
    assert CT == sum(cpad)

    # Pair-ordered token index list (padding repeats token 0; its output is
    # ignored at combine time).
    tokens = np.zeros(CT, dtype=np.int64)
    for e in range(E):
        tokens[exp_off[e] : exp_off[e] + counts[e]] = idx_per_e[e]

    nc = _build_program(D, HL, O, E, chunks, CT)

    # x in flat per-partition chunk-major consumption order (shared by all
    # cores): [128p, chunk, kd, col].
    xg = x[tokens].astype(ml_dtypes.bfloat16)  # [CT, D]
    xparts = []
    for e, N, c0 in chunks:
        seg = xg[c0 : c0 + N].T  # [D, N]
        xparts.append(seg.reshape(KD, P, N).transpose(1, 0, 2).reshape(P, KD * N))
    xTp = np.ascontiguousarray(np.concatenate(xparts, axis=1))

    in_maps = []
    for s in range(N_CORES):
        w1ps, w2ps, b1ps, b2ps = [], [], [], []
        sl = slice(s * HL, (s + 1) * HL)
        for e in range(E):
            w1ps.append(
                W1[e][:, sl]
                .reshape(KD, P, MHL, P)
                .transpose(1, 2, 0, 3)
                .reshape(P, MHL * KD * P)
            )
            w2ps.append(
                W2[e][sl, :]
                .reshape(MHL, P, MO, P)
                .transpose(1, 2, 0, 3)
                .reshape(P, MO * MHL * P)
            )
            b1ps.append(b1[e, sl].reshape(MHL, P).T)
            b2ps.append((b2[e] / N_CORES).reshape(MO, P).T)
        in_maps.append(
            {
                "xTp": xTp,
                "w1p": np.ascontiguousarray(
                    np.concatenate(w1ps, axis=1).astype(ml_dtypes.bfloat16)
                ),
                "w2p": np.ascontiguousarray(
                    np.concatenate(w2ps, axis=1).astype(ml_dtypes.bfloat16)
                ),
                "b1p": np.ascontiguousarray(np.concatenate(b1ps, axis=1)),
                "b2p": np.ascontiguousarray(np.concatenate(b2ps, axis=1)),
            }
        )

    res = run_bass_kernel_spmd(nc, in_maps, core_ids=list(range(N_CORES)))
    globals()["_last_results"] = res

    # Sum the 8 hidden-slice partials, then the weighted scatter-add combine.
    ysum = np.zeros((P, MO * CT), dtype=np.float32)
    for s in range(N_CORES):
        ysum += res.results[s]["yT"].astype(np.float32)
    # decode chunk-contiguous blocks: [P, MO, N] per chunk -> [col, mo*P+p]
    y_full = np.empty((CT, O), dtype=np.float32)
    for e, N, c0 in chunks:
        blk = ysum[:, MO * c0 : MO * (c0 + N)].reshape(P, MO, N)
        y_full[c0 : c0 + N] = blk.transpose(2, 1, 0).reshape(N, O)

    out = np.zeros((B, O), dtype=np.float32)
    for e in range(E):
        cnt = counts[e]
        if cnt:
            idx = idx_per_e[e]
            c0 = exp_off[e]
            out[idx] += probs[idx, e : e + 1] * y_full[c0 : c0 + cnt]
    return out


# revision 32
# speedup vs baseline: 1.0554x; 1.0554x over previous
"""MoE layer (router + top-k dispatch + per-expert FFN + weighted combine)
on 8 Trainium2 NeuronCores.

Sharding strategy (hidden-dim sharding = perfectly balanced expert
parallelism):
  - The host computes the router (x @ Wg -> softmax -> top-k) and builds
    ONE global pair list: all routed (token, expert) pairs, grouped by
    expert, each expert's list padded to a multiple of 8.  Every core
    walks the SAME pair list (same program, same chunk structure - clean
    SPMD), but core s only owns the s-th H/8 = 512-wide slice of every
    expert's FFN: W1[e][:, s*512:(s+1)*512] and W2[e][s*512:(s+1)*512, :].
    ReLU is elementwise in H, so slicing the hidden dim is exact; each
    core produces a partial y and the host sums the 8 partials during the
    weighted combine.  Per-core work is exactly sum(counts)/8 = B*k/8
    token-equivalents regardless of how unbalanced the expert counts are
    (expert-per-core would be bound by max(counts)).
  - Device output yT[p, mo, c] (fp16 partials) is summed + combined with
    the fp32 softmax weights host-side (the "weighted return" half of the
    expert-parallel all-to-all).

Device program: per 512-token chunk, GEMM1 (4 psum groups x 8 k-matmuls)
-> relu+bias eviction to bf16 h, GEMM2 (8 psum groups x 4 k-matmuls) ->
bias eviction to fp16 ob -> one batched DMA to DRAM.  The two GEMMs are
software-pipelined one chunk deep (PE order G1(0), G1(1), G2(0), G1(2),
G2(1), ...) so GEMM2 never waits on the tail h eviction of its own
chunk's GEMM1 and the cold-start w2 deadline is one extra chunk out.

Layouts: host pre-permutes everything into flat per-partition consumption
order (w1 as [128p, e, mh, kd, 128q], w2 as [128p, e, mo, kh, 128q], x as
[128p, chunk, kd, col]) so every DMA is a flat contiguous slice arriving
in consumption order.  Weight streaming beyond expert 0 is paced one
segment ahead with a data-dependency gate (a 1-element op reading the
pacing chunk's h and writing into the DMA's destination) — the tile
scheduler hoists dep-free DMA triggers to the top of their queue, so
queue position alone cannot hold a transfer back.  Stores ride the
scalar hardware-DGE queue; x rides sync; the gpsimd software-DGE queue
only carries cold-start weights, paced weight prefetch, and tail stores.

Compute is bf16 (fp32 PSUM accumulation); combine weights stay fp32;
partial y ships as fp16.
"""

import numpy as np
import ml_dtypes
import bass_rust

import concourse.bass as bass
import concourse.mybir as mybir
import concourse.tile as tile
from concourse.bass_utils import run_bass_kernel_spmd

P = 128
N_CORES = 8
CHUNK = 512
MIN_CHUNK = 256  # below ~230 cols LDWEIGHTS (97ns) outruns the matmul


def _normalize_sync_waits(nc):
    """The walrus build in this toolchain rejects >1 sync wait on a single
    instruction (setupSyncWait: "Too many sync wait commands"), while Tile's
    semaphore assignment freely emits several. Hoist all but one wait of each
    instruction onto same-engine NOPs placed immediately before it — the
    engine stream is in-order, so stalling at the NOPs is semantically
    identical to a multi-wait instruction."""
    count = 0
    for f in nc.m.functions:
        for bb in f.blocks:
            out = []
            changed = False
            for ins in bb.instructions:
                si = ins.sync_info
                if si is not None and len(si.on_wait) > 1:
                    waits = list(si.on_wait)
                    for w in waits[:-1]:
                        count += 1
                        out.append(
                            mybir.InstNoOp(
                                name=f"I-nw{count}",
                                ins=[],
                                outs=[],
                                engine=ins.engine,
                                sync_info=bass_rust.SyncInfo(
                                    on_wait=[w], on_update=[]
                                ),
                            )
                        )
                    ins.sync_info = bass_rust.SyncInfo(
                        on_wait=[waits[-1]], on_update=list(si.on_update)
                    )
                    changed = True
                out.append(ins)
            if changed:
                bb.instructions = out
    return nc


def _build_program(D, HL, O, E, chunks, CT):
    """chunks: list of (expert, N, c0) covering [0, CT) in order."""
    f32, bf16, f16 = mybir.dt.float32, mybir.dt.bfloat16, mybir.dt.float16
    KD, MHL, MO = D // P, HL // P, O // P
    AF = mybir.ActivationFunctionType
    G1 = KD * P        # w1 cols per GEMM1 group
    W1E = MHL * G1     # w1 cols per expert slice
    W2E = MO * MHL * P # w2 cols per expert slice

    nc = bass.Bass()
    xTp = nc.declare_dram_parameter("xTp", [P, KD * CT], bf16, isOutput=False)
    w1p = nc.declare_dram_parameter("w1p", [P, E * W1E], bf16, isOutput=False)
    w2p = nc.declare_dram_parameter("w2p", [P, E * W2E], bf16, isOutput=False)
    b1p = nc.declare_dram_parameter("b1p", [P, E * MHL], f32, isOutput=False)
    b2p = nc.declare_dram_parameter("b2p", [P, E * MO], f32, isOutput=False)
    # Output blocks are chunk-contiguous ([mo, col] within each chunk's
    # MO*N-wide block) so every store is one flat 8KB-per-partition-row DMA
    # on a hardware-DGE queue - the strided [P, MO, CT] layout fed through
    # the gpsimd software-DGE queue ran at ~50 GB/s and stalled the whole
    # pipeline through the ob WAR chain.
    yT = nc.declare_dram_parameter("yT", [P, MO * CT], f16, isOutput=True)

    # order of first appearance of each expert in the chunk list
    seg_experts = []
    for e, _, _ in chunks:
        if e not in seg_experts:
            seg_experts.append(e)
    first_chunk_of = {}
    n_chunks_of = {}
    for ci, (e, _, _) in enumerate(chunks):
        if e not in first_chunk_of:
            first_chunk_of[e] = ci
        n_chunks_of[e] = n_chunks_of.get(e, 0) + 1
    # Chunk at which to prefetch the NEXT segment's weights: two chunks into
    # the previous segment (capped at its last chunk) — far enough from the
    # cold-start window that the trigger doesn't steal wire from chunk 0-1's
    # x/w stream, and still >=1.5 chunk periods (~20us) of lead for ~6us of
    # wire.
    # prefetch_at: chunk index -> list of (expert, w1?) gates to emit there.
    # w1 (needed first) gets one chunk more lead than w2; both are paced by
    # a data-dependency gate so jitter headroom is ~2-4 chunk periods.
    prefetch_at = {}
    # expert 0's own w2 is also gated (on chunk 0's first h): it is not
    # consumed until GEMM2(0) at ~+10us, and pulling its 1MB out of the
    # cold window frees wire for the chunk 0-2 x / w1 stream.
    if seg_experts:
        prefetch_at.setdefault(0, []).append((seg_experts[0], False))
    for j in range(1, len(seg_experts)):
        ep = seg_experts[j - 1]
        en = seg_experts[j]
        ci_w1 = first_chunk_of[ep] + min(1, n_chunks_of[ep] - 1)
        ci_w2 = first_chunk_of[ep] + min(2, n_chunks_of[ep] - 1)
        prefetch_at.setdefault(ci_w1, []).append((en, True))
        prefetch_at.setdefault(ci_w2, []).append((en, False))

    with tile.TileContext(nc) as tc:
        with (
            tc.tile_pool(name="sb", bufs=1) as pool,
            tc.tile_pool(name="ps", bufs=4, space="PSUM") as psp,
        ):
            w1_sb = pool.tile([P, E * W1E], bf16)
            w2_sb = pool.tile([P, E * W2E], bf16)
            b1_sb = pool.tile([P, E * MHL], f32)
            b2_sb = pool.tile([P, E * MO], f32)

            e0, N0, _ = chunks[0]

            # Hoist the scalar engine's one-time ACT_TABLE_LOAD (~1.3us) into
            # the cold DMA window with a dependency-free dummy activation.
            dum_i = pool.tile([P, 1], f32, tag="dum_i")
            dum_o = pool.tile([P, 1], f32, tag="dum_o")
            nc.vector.memset(dum_i[:], 0.0)
            nc.scalar.activation(dum_o[:], dum_i[:], AF.Relu)
            # Warm the PE p-state during the cold DMA window: ~3us of dummy
            # matmuls (dep-free, so the scheduler runs them immediately)
            # bring the clock to max before the first real matmul — a cold
            # PE runs the first ~3us of matmuls at 0.65-1.2GHz instead of
            # 2.4GHz.
            dum_w = pool.tile([P, P], bf16, tag="dum_w")
            dum_x = pool.tile([P, CHUNK], bf16, tag="dum_x")
            nc.vector.memset(dum_w[:], 0.0)
            nc.vector.memset(dum_x[:], 0.0)
            dum_ps = psp.tile([P, CHUNK], f32, tag="ph")
            for i in range(11):
                nc.tensor.matmul(
                    dum_ps[:], dum_w[:], dum_x[:],
                    start=(i == 0), stop=(i == 10),
                )

            # Cold start: b1 first (needed at the first eviction), then the
            # chunk-0 x slices and expert-0 weights as flat consumption-order
            # slices.  x rides the sync queue, weights the gpsimd queue; the
            # two streams split the wire, and chunk-0's GEMM1 group g only
            # waits on its own slice (Tile RAW deps are per-DMA-region), so
            # w1 goes group-by-group.
            nc.scalar.dma_start(b1_sb[:], b1p[:])
            xc0 = pool.tile([P, KD * CHUNK], bf16, tag="xc", bufs=2)
            # finest-grain first slices: MM0 only needs x[kd0] + w1[g0,kd0]
            nc.sync.dma_start(xc0[:, :N0], xTp[:, :N0])
            nc.gpsimd.dma_start(
                w1_sb[:, e0 * W1E : e0 * W1E + P],
                w1p[:, e0 * W1E : e0 * W1E + P],
            )
            nc.sync.dma_start(xc0[:, N0 : 3 * N0], xTp[:, N0 : 3 * N0])
            nc.gpsimd.dma_start(
                w1_sb[:, e0 * W1E + P : e0 * W1E + G1],
                w1p[:, e0 * W1E + P : e0 * W1E + G1],
            )
            nc.sync.dma_start(xc0[:, 3 * N0 : 5 * N0], xTp[:, 3 * N0 : 5 * N0])
            nc.sync.dma_start(xc0[:, 5 * N0 : KD * N0], xTp[:, 5 * N0 : KD * N0])
            for g in range(1, MHL):
                nc.gpsimd.dma_start(
                    w1_sb[:, e0 * W1E + g * G1 : e0 * W1E + (g + 1) * G1],
                    w1p[:, e0 * W1E + g * G1 : e0 * W1E + (g + 1) * G1],
                )
            nc.scalar.dma_start(b2_sb[:], b2p[:])

            xcs = {0: xc0}
            hs = {}
            last_ci = len(chunks) - 1

            def emit_g1(ci):
                e, N, c0 = chunks[ci]
                # prefetch next chunk's x (sync queue; self-paced by the
                # bufs=2 WAR dependency after the first two chunks)
                if ci + 1 < len(chunks):
                    e1, N1, c1 = chunks[ci + 1]
                    xn = pool.tile([P, KD * CHUNK], bf16, tag="xc", bufs=2)
                    nc.sync.dma_start(
                        xn[:, : KD * N1], xTp[:, KD * c1 : KD * (c1 + N1)]
                    )
                    xcs[ci + 1] = xn
                xc = xcs.pop(ci)
                pf = prefetch_at.get(ci)
                h_a = pool.tile([P, MHL - MHL // 2, CHUNK], bf16, tag="h_a", bufs=2)
                h_b = pool.tile([P, MHL // 2, CHUNK], bf16, tag="h_b", bufs=2)
                hs[ci] = (h_a, h_b)

                def h_slice(kh, N=N, h_a=h_a, h_b=h_b):
                    half = MHL - MHL // 2
                    t = h_a if kh < half else h_b
                    return t[:, kh if kh < half else kh - half, :N]

                for mh in range(MHL):
                    ph = psp.tile([P, CHUNK], f32, tag="ph")
                    for kd in range(KD):
                        u = (e * MHL + mh) * KD + kd
                        nc.tensor.matmul(
                            ph[:, :N],
                            w1_sb[:, u * P : (u + 1) * P],
                            xc[:, kd * N : (kd + 1) * N],
                            start=(kd == 0),
                            stop=(kd == KD - 1),
                        )
                    # h eviction on the otherwise-idle DVE (relu(ph + b1) via
                    # tensor_scalar) keeps the scalar queue for GEMM2
                    # evictions + DMA triggers — scalar saturation here
                    # stalled GEMM2 at segment boundaries.
                    nc.vector.tensor_scalar(
                        h_slice(mh), ph[:, :N],
                        b1_sb[:, e * MHL + mh : e * MHL + mh + 1], 0.0,
                        mybir.AluOpType.add, mybir.AluOpType.max,
                    )
                    if pf is not None and mh == 0:
                        # Pace the next segment's weight stream behind THIS
                        # chunk's compute with a real data dependency — the
                        # tile scheduler freely hoists dep-free DMA triggers
                        # to the top of their queue (observed: all prefetches
                        # fired inside the cold window, starving chunk 0-1's
                        # x stream).  A 1-element gate op reads this chunk's
                        # freshly-evicted h (RAW on compute) and writes into
                        # each DMA's destination region (WAW with the DMA).
                        for en, is_w1 in pf:
                            wsb, wpr, sz = (
                                (w1_sb, w1p, W1E) if is_w1 else (w2_sb, w2p, W2E)
                            )
                            nc.vector.tensor_scalar(
                                wsb[:, en * sz : en * sz + 1],
                                h_slice(0)[:, 0:1], 0.0, None,
                                mybir.AluOpType.mult,
                            )
                            nc.gpsimd.dma_start(
                                wsb[:, en * sz : (en + 1) * sz],
                                wpr[:, en * sz : (en + 1) * sz],
                            )

            def emit_g2(ci):
                e, N, c0 = chunks[ci]
                h_a, h_b = hs.pop(ci)
                half = MHL - MHL // 2

                def h_slice(kh):
                    t = h_a if kh < half else h_b
                    return t[:, kh if kh < half else kh - half, :N]

                ob = pool.tile([P, MO, CHUNK], f16, tag="ob", bufs=4)
                for mo in range(MO):
                    py = psp.tile([P, CHUNK], f32, tag="py")
                    for kh in range(MHL):
                        u = (e * MO + mo) * MHL + kh
                        nc.tensor.matmul(
                            py[:, :N],
                            w2_sb[:, u * P : (u + 1) * P],
                            h_slice(kh),
                            start=(kh == 0),
                            stop=(kh == MHL - 1),
                        )
                    if ci >= last_ci - 1:
                        # tail-critical final chunks: evictions move to the
                        # end-idle Vector engine (bias-add via tensor_scalar)
                        # so the scalar queue only carries store triggers,
                        # and each mo group streams out immediately on the
                        # sync/scalar hardware-DGE queues (gpsimd SW-DGE
                        # costs ~1us fixed per store and dominated the
                        # drain; extra triggers on a busy scalar stalled
                        # the PE before that).
                        nc.vector.tensor_scalar(
                            ob[:, mo, :N], py[:, :N],
                            b2_sb[:, e * MO + mo : e * MO + mo + 1], None,
                            mybir.AluOpType.add,
                        )
                        eng = nc.sync if mo % 2 == 0 else nc.scalar
                        eng.dma_start(
                            yT[:, MO * c0 + mo * N : MO * c0 + (mo + 1) * N],
                            ob[:, mo, :N],
                        )
                    else:
                        nc.scalar.activation(
                            ob[:, mo, :N], py[:, :N], AF.Identity,
                            bias=b2_sb[:, e * MO + mo : e * MO + mo + 1],
                        )
                if ci < last_ci - 1:
                    # one flat store per chunk (scalar queue = hardware DGE,
                    # naturally ordered after the mo=7 eviction)
                    nc.scalar.dma_start(
                        yT[:, MO * c0 : MO * (c0 + N)], ob[:, :, :N]
                    )

            # software pipeline: G1(0), G1(1), G2(0), G1(2), G2(1), ...
            emit_g1(0)
            for ci in range(len(chunks)):
                if ci + 1 < len(chunks):
                    emit_g1(ci + 1)
                emit_g2(ci)
    return _normalize_sync_waits(nc)


def _chunk_counts(c):
    """Split a padded per-expert count into chunks <=CHUNK, all >=MIN_CHUNK
    (borrowing from the previous full chunk when the tail is short)."""
    ch = [CHUNK] * (c // CHUNK)
    rem = c % CHUNK
    if rem:
        if rem < MIN_CHUNK and ch:
            ch[-1] -= MIN_CHUNK - rem
            rem = MIN_CHUNK
        ch.append(rem)
    return ch


def kernel(**inputs):
    x = np.ascontiguousarray(np.asarray(inputs["x"], dtype=np.float32))
    Wg = np.ascontiguousarray(np.asarray(inputs["Wg"], dtype=np.float32))
    W1 = np.asarray(inputs["W1"], dtype=np.float32)
    b1 = np.asarray(inputs["b1"], dtype=np.float32)
    W2 = np.asarray(inputs["W2"], dtype=np.float32)
    b2 = np.asarray(inputs["b2"], dtype=np.float32)
    k = int(np.asarray(inputs["k"]))

    B, D = x.shape
    E = Wg.shape[1]
    H = W1.shape[2]
    O = W2.shape[2]
    HL = H // N_CORES              # hidden slice per core
    KD, MHL, MO = D // P, HL // P, O // P

    # Host-side router: softmax probs (combine weights) and top-k expert
    # membership (softmax is monotonic, so top-k on logits == on probs).
    logits = x @ Wg
    m = logits.max(axis=1, keepdims=True)
    p = np.exp(logits - m)
    probs = p / p.sum(axis=1, keepdims=True)
    kth = np.partition(logits, E - k, axis=1)[:, E - k]
    routed = logits >= kth[:, None]
    idx_per_e = [np.nonzero(routed[:, e])[0] for e in range(E)]
    counts = [len(ix) for ix in idx_per_e]
    cpad = [-(-c // 2) * 2 for c in counts]

    # Global chunk list over the pair list (grouped by expert).  The first
    # two chunks are 256-col (halves the cold-start x wire ahead of the
    # first GEMMs) and the global last chunk is 256-col (shrinks the tail
    # store drain).
    active = [e for e in range(E) if cpad[e] > 0]
    chunks = []   # (expert, N, global col offset)
    exp_off = [0] * E
    CT = 0
    for e in range(E):
        exp_off[e] = CT
        c = cpad[e]
        if active and e == active[0] and c >= CHUNK + 2 * MIN_CHUNK:
            ch = [MIN_CHUNK, MIN_CHUNK] + _chunk_counts(c - 2 * MIN_CHUNK)
        elif active and e == active[-1] and c >= CHUNK + 3 * MIN_CHUNK:
            ch = _chunk_counts(c - 2 * MIN_CHUNK) + [MIN_CHUNK, MIN_CHUNK]
        else:
            ch = _chunk_counts(c)
        for N in ch:
            chunks.append((e, N, CT))
            CT += N
    assert CT == sum(cpad)

    # Pair-ordered token index list (padding repeats token 0; its output is
    # ignored at combine time).
    tokens = np.zeros(CT, dtype=np.int64)
    for e in range(E):
        tokens[exp_off[e] : exp_off[e] + counts[e]] = idx_per_e[e]

    nc = _build_program(D, HL, O, E, chunks, CT)

    # x in flat per-partition chunk-major consumption order (shared by all
    # cores): [128p, chunk, kd, col].
    xg = x[tokens].astype(ml_dtypes.bfloat16)  # [CT, D]
    xparts = []
    for e, N, c0 in chunks:
        seg = xg[c0 : c0 + N].T  # [D, N]
        xparts.append(seg.reshape(KD, P, N).transpose(1, 0, 2).reshape(P, KD * N))
    xTp = np.ascontiguousarray(np.concatenate(xparts, axis=1))

    in_maps = []
    for s in range(N_CORES):
        w1ps, w2ps, b1ps, b2ps = [], [], [], []
        sl = slice(s * HL, (s + 1) * HL)
        for e in range(E):
            w1ps.append(
                W1[e][:, sl]
                .reshape(KD, P, MHL, P)
                .transpose(1, 2, 0, 3)
                .reshape(P, MHL * KD * P)
            )
            w2ps.append(
                W2[e][sl, :]
                .reshape(MHL, P, MO, P)
                .transpose(1, 2, 0, 3)
                .reshape(P, MO * MHL * P)
            )
            b1ps.append(b1[e, sl].reshape(MHL, P).T)
            b2ps.append((b2[e] / N_CORES).reshape(MO, P).T)
        in_maps.append(
            {
                "xTp": xTp,
                "w1p": np.ascontiguousarray(
                    np.concatenate(w1ps, axis=1).astype(ml_dtypes.bfloat16)
                ),
                "w2p": np.ascontiguousarray(
                    np.concatenate(w2ps, axis=1).astype(ml_dtypes.bfloat16)
                ),
                "b1p": np.ascontiguousarray(np.concatenate(b1ps, axis=1)),
                "b2p": np.ascontiguousarray(np.concatenate(b2ps, axis=1)),
            }
        )

    res = run_bass_kernel_spmd(nc, in_maps, core_ids=list(range(N_CORES)))
    globals()["_last_results"] = res

    # Sum the 8 hidden-slice partials, then the weighted scatter-add combine.
    ysum = np.zeros((P, MO * CT), dtype=np.float32)
    for s in range(N_CORES):
        ysum += res.results[s]["yT"].astype(np.float32)
    # decode chunk-contiguous blocks: [P, MO, N] per chunk -> [col, mo*P+p]
    y_full = np.empty((CT, O), dtype=np.float32)
    for e, N, c0 in chunks:
        blk = ysum[:, MO * c0 : MO * (c0 + N)].reshape(P, MO, N)
        y_full[c0 : c0 + N] = blk.transpose(2, 1, 0).reshape(N, O)

    out = np.zeros((B, O), dtype=np.float32)
    for e in range(E):
        cnt = counts[e]
        if cnt:
            idx = idx_per_e[e]
            c0 = exp_off[e]
            out[idx] += probs[idx, e : e + 1] * y_full[c0 : c0 + cnt]
    return out
